# revision 1
# baseline (speedup 1.0000x reference)
"""NVFP4-style activation quantizer on 8 TRN2 NeuronCores (raw bass).

Reference semantics (per 16-element block, fp32):
    s_t  = max|x| / (6*448)                      (global, needs all-reduce)
    m_b  = max|x| over block
    inv  = 6 / (m_b / s_t)
    s_b  = fp8_e4m3_roundtrip(inv)   (the 0/inf guard is dead code for this
                                      input: inv >= 6/2688 = 2.23e-3 > 2^-10)
    out  = sign(x) * fp4_121(|x|/s_t * s_b) / s_b * s_t

All-16-bit quantize chain (measured rel_l2 vs reference: 1.05e-2, well
under the 2e-2 gate).  The fp4_121 magic-add works in fp16: the grid
step of the 1-2-1 code is ulp16(768 * max(2^e(y),1)), so

    y16 = x16 * c16                  (fp16 TT, 2x mode w/ dense c16)
    p   = bits16(y) & 0x7C00         (int16 TS, 4x mode)
    Bb  = max(p + 0x2600, 0x6200)    (int16 TS, 4x)  -> bits of 768*2^k
    t   = y + B                      (fp16 TT, 2x; internal fp32, RNE out)
    nq  = B - t                      (fp16 TT, 2x; = -fp4_121(y), exact)
    o   = nq * nic                   (TT vs fp16-broadcast nic, 1x, fp32 out)

~3.3 DVE cycles/element vs ~6 for the fp32 chain.  ScalarE feeds it:
per-tile fp32->fp16 conversion of x, per-tile materialization of the
per-block c16 into a dense unit-stride fp16 stream (the 2x TT mode
forfeits on stride-0 broadcasts), and both reciprocal families of the
scale chain (spline Reciprocal, accurate to ~1e-7, emitted directly
since the bass wrapper refuses it; 1/m runs in place over the
block-max buffer, gated on per-quarter partial maxes).

HBM traffic: 1.5 reads + 1 write of x.  Tiles 0..TC-1 are cached in
SBUF as fp16 during pass A; tiles TC.. are re-read in pass B into the
ring slot their predecessor tile vacated, with conversions running 3
tiles ahead of the consumer.  The first re-reads prefetch inside the
AllReduce dead window.  Input DMAs are split across the SYNC and ACT
hardware queues; pass-B re-reads issue from the otherwise-idle GPSIMD
queue so the SYNC queue carries only output traffic.  A micro first
scale-chunk (128 blocks) lets pass B start ~7us after the AllReduce.
GPSIMD otherwise runs only the pre-warmed AllReduce.

Engine occupancy on HW: DVE ~310us busy (92%+ of pass windows),
ScalarE ~140us, AllReduce window 15-95us (run-to-run variance of the
tunneled collective), input read ~125us at the ~270-360 GB/s the
fabric delivers.
"""

import numpy as np

FULL_SHAPE = (4, 4096, 4096)
N_CORES = 8
P = 128
TOTAL = 4 * 4096 * 4096
L = TOTAL // (N_CORES * P)   # 65536 elements per partition per core
NBLK = L // 16

F = 2048
T = L // F                   # 32 tiles
TC = 16                      # tiles cached as fp16 during pass A
NQ = 4                       # scale-chain quarters
FBLK = F // 16
QBLK = NBLK // NQ
TQ = T // NQ                 # tiles per quarter (quarter q covers 8 tiles)
N_XA = 4
N_CF = 3
N_OB = 2
EARLY_RR = 4                 # re-reads prefetched during the AR window

H_EXPMASK = 0x7C00
H_MAGIC_ADD = 0x2600
H_MAGIC_MIN = 0x6200

# scale-chain chunks (in blocks): a micro first chunk lets the first
# cfull/y start ~7us earlier after the AllReduce
CHUNKS = [128, 896, 1024, 1024, 1024]
CH_START = [sum(CHUNKS[:i]) for i in range(len(CHUNKS))]
NCH = len(CHUNKS)


def chunk_of_tile(t):
    b = t * FBLK
    for c in range(NCH):
        if CH_START[c] <= b < CH_START[c] + CHUNKS[c]:
            return c
    raise ValueError(t)


CH_FIRST_TILE = {}
for _t in range(T - 1, -1, -1):
    CH_FIRST_TILE[chunk_of_tile(_t)] = _t
CH_GATE = {v: k for k, v in CH_FIRST_TILE.items()}  # first tile -> chunk


def _plan_xa():
    """Order of DMAs into the xa slots; returns per-tile (slot, sem count)
    and the previous tile in the same slot (whose consumers gate reuse)."""
    order = list(range(T)) + list(range(TC, T))   # pass A, then re-reads
    count = [0] * N_XA
    need = {}
    prev = {}
    last = [None] * N_XA
    for i, t in enumerate(order):
        s = t % N_XA
        count[s] += 1
        key = (t, i >= T)
        need[key] = (s, 16 * count[s])
        prev[key] = last[s]
        last[s] = key
    return need, prev


XA_NEED, XA_PREV = _plan_xa()


def build_nc(n_cores=N_CORES):
    from contextlib import ExitStack

    import concourse.bass as bass
    from concourse import mybir

    f32 = mybir.dt.float32
    f16 = mybir.dt.float16
    i16 = mybir.dt.int16
    f8 = mybir.dt.float8e4

    nc = bass.Bass(num_devices=n_cores, debug=False)
    x_ext = nc.declare_dram_parameter("x", [P, L], f32, isOutput=False)
    out_ext = nc.declare_dram_parameter("out", [P, L], f32, isOutput=True)
    cc_in = nc.dram_tensor("cc_in", [1, 128], f32)
    cc_out = nc.dram_tensor("cc_out", [1, 128], f32, addr_space="Shared")
    cc_warm_in = nc.dram_tensor("cc_warm_in", [1, 128], f32)
    cc_warm_out = nc.dram_tensor("cc_warm_out", [1, 128], f32,
                                 addr_space="Shared")

    def act_reciprocal(act, out, in_):
        return act.add_instruction(
            mybir.InstActivation(
                name=act.bass.get_next_instruction_name(),
                func=mybir.ActivationFunctionType.Reciprocal,
                ins=[
                    act.lower_ap(in_),
                    mybir.ImmediateValue(dtype=f32, value=0.0),
                    mybir.ImmediateValue(dtype=f32, value=1.0),
                    mybir.ImmediateValue(dtype=f32, value=0.0),
                ],
                outs=[act.lower_ap(out)],
            )
        )

    with ExitStack() as ctx:
        def sem(name):
            return ctx.enter_context(nc.semaphore(name))

        def sbuf(name, shape, dt=f32):
            return ctx.enter_context(nc.sbuf_tensor(name, shape, dt))

        s_xa = [sem(f"s_xa{i}") for i in range(N_XA)]
        s_ob = [sem(f"s_ob{i}") for i in range(N_OB)]
        s_cdma = sem("s_cdma")
        s_dve = sem("s_dve")
        s_act = sem("s_act")     # ACT fp16 converts (+1, in tile order)
        s_cf = sem("s_cf")       # ACT cfull materializations (+1)
        s_acr = sem("s_acr")     # ACT reciprocals (+1)
        s_cc = sem("s_cc")
        s_pool = sem("s_pool")
        s_warm = sem("s_warm")

        # fp16 tile cache used as a ring: pass A fills slots 0..TC-1 with
        # tiles 0..TC-1; in pass B, re-read tile TC+k converts into slot k
        # once y(k) has consumed it.
        xh = sbuf("xh", [P, TC * F], f16)
        xa = [sbuf(f"xa{i}", [P, F]) for i in range(N_XA)]
        y16 = [sbuf(f"y16_{i}", [P, F], f16) for i in range(2)]
        pb16 = [sbuf(f"pb16_{i}", [P, F], i16) for i in range(2)]
        t16 = [sbuf(f"t16_{i}", [P, F], f16) for i in range(2)]
        nq16 = [sbuf(f"nq16_{i}", [P, F], f16) for i in range(2)]
        cfull = [sbuf(f"cfull{i}", [P, F], f16) for i in range(N_CF)]
        ob = [sbuf(f"ob{i}", [P, F]) for i in range(N_OB)]
        m_t = sbuf("m_t", [P, NBLK])     # blockmax -> 1/m (in place) -> s_b
        rs2 = [sbuf(f"rs2_{i}", [P, QBLK]) for i in range(2)]
        f8_t = sbuf("f8_t", [P, QBLK], f8)
        c16_t = sbuf("c16_t", [P, NBLK], f16)
        nic16_t = sbuf("nic16_t", [P, NBLK], f16)
        gall_t = sbuf("gall_t", [P, 128])
        mxq_t = sbuf("mxq_t", [P, NQ])
        mx_t = sbuf("mx_t", [P, 1])
        g128_t = sbuf("g128_t", [P, 1])
        st_t = sbuf("st_t", [P, 1])
        rt_t = sbuf("rt_t", [P, 1])
        k6_t = sbuf("k6_t", [P, 1])
        nst_t = sbuf("nst_t", [P, 1])

        dveA = [0] * T
        K_mxq = [0] * NQ
        tag_y = [0] * T
        tag_nq = [0] * T
        tag_o = [0] * T
        K_mx = [0]
        K_sb = [0] * NCH
        K_c = [0] * NCH
        K_nic = [0] * NCH

        def b3(ap):
            return ap.rearrange("p (b s) -> p b s", s=16)

        def qs(q):
            return slice(q * QBLK, (q + 1) * QBLK)

        def conv_done(t):
            """s_act value after conv(t): convs run in tile order
            0..TC-1 (pass A) then TC..T-1."""
            return t + 1

        with nc.Block() as block:

            @block.vector
            def _(dve):
                cnt = 0

                def tag(ins):
                    nonlocal cnt
                    ins.then_inc(s_dve)
                    cnt += 1
                    return cnt

                # ---- pass A: per-block abs max ----
                for t in range(T):
                    dve.wait_ge(s_xa[XA_NEED[(t, False)][0]],
                                XA_NEED[(t, False)][1])
                    dveA[t] = tag(dve.tensor_reduce(
                        out=m_t[:, t * FBLK:(t + 1) * FBLK],
                        in_=b3(xa[t % N_XA][:]),
                        axis=mybir.AxisListType.X,
                        op=mybir.AluOpType.max,
                        apply_absolute_value=True,
                    ))
                    if (t + 1) % TQ == 0:
                        # partial max of this m-quarter, so ACT's in-place
                        # 1/m can start before the global reduce
                        q = t // TQ
                        dve.wait_ge(s_dve, dveA[t])
                        K_mxq[q] = tag(dve.tensor_reduce(
                            out=mxq_t[:, q:q + 1], in_=m_t[:, qs(q)],
                            axis=mybir.AxisListType.X,
                            op=mybir.AluOpType.max,
                        ))
                dve.wait_ge(s_dve, K_mxq[NQ - 1])
                K_mx[0] = tag(dve.tensor_reduce(
                    out=mx_t[:], in_=mxq_t[:], axis=mybir.AxisListType.X,
                    op=mybir.AluOpType.max,
                ))

                # ---- global scalars (post-AllReduce) ----
                dve.wait_ge(s_cdma, 32)
                k = tag(dve.tensor_reduce(
                    out=g128_t[:], in_=gall_t[:], axis=mybir.AxisListType.X,
                    op=mybir.AluOpType.max))
                dve.wait_ge(s_dve, k)
                k_st = tag(dve.tensor_scalar(
                    st_t[:], g128_t[:], 1.0 / 2688.0, None,
                    op0=mybir.AluOpType.mult))
                dve.wait_ge(s_dve, k_st)
                k_rt = tag(dve.reciprocal(rt_t[:], st_t[:]))
                k_k6 = tag(dve.tensor_scalar(
                    k6_t[:], st_t[:], 6.0, None, op0=mybir.AluOpType.mult))
                k_nst = tag(dve.tensor_scalar(
                    nst_t[:], st_t[:], -1.0, None, op0=mybir.AluOpType.mult))
                dve.wait_ge(s_dve, k_nst)

                # ---- per-block scales, in CHUNKS (rm already in m_t) ----
                # chunk c needs rm coverage: chunk->rm-quarter wait
                rmq = [1, 1, 2, 3, 4]
                for c in range(NCH):
                    cs = slice(CH_START[c], CH_START[c] + CHUNKS[c])
                    n = CHUNKS[c]
                    dve.wait_ge(s_acr, rmq[c])
                    k_f8 = tag(dve.tensor_scalar(
                        f8_t[:, 0:n], m_t[:, cs], k6_t[:], None,
                        op0=mybir.AluOpType.mult))
                    dve.wait_ge(s_dve, k_f8)
                    K_sb[c] = tag(dve.tensor_copy(m_t[:, cs], f8_t[:, 0:n]))
                    dve.wait_ge(s_dve, K_sb[c])
                    K_c[c] = tag(dve.tensor_scalar(
                        c16_t[:, cs], m_t[:, cs], rt_t[:], None,
                        op0=mybir.AluOpType.mult))
                    if c >= 1:
                        pc = c - 1
                        pcs = slice(CH_START[pc], CH_START[pc] + CHUNKS[pc])
                        dve.wait_ge(s_acr, NQ + pc + 1)
                        K_nic[pc] = tag(dve.tensor_scalar(
                            nic16_t[:, pcs], rs2[pc % 2][:, 0:CHUNKS[pc]],
                            nst_t[:], None, op0=mybir.AluOpType.mult))
                dve.wait_ge(s_acr, NQ + NCH)
                lc = NCH - 1
                lcs = slice(CH_START[lc], CH_START[lc] + CHUNKS[lc])
                K_nic[lc] = tag(dve.tensor_scalar(
                    nic16_t[:, lcs], rs2[lc % 2][:, 0:CHUNKS[lc]],
                    nst_t[:], None, op0=mybir.AluOpType.mult))

                # ---- pass B: 16-bit quantize chain, pairs of tiles ----
                tag_pb = [0] * T
                tag_t = [0] * T
                for tp in range(0, T, 2):
                    pair = (tp, tp + 1)
                    for t in pair:
                        if t >= 2:
                            dve.wait_ge(s_dve, tag_o[t - 2])
                        if t in CH_GATE:
                            dve.wait_ge(s_dve, K_c[CH_GATE[t]])
                        dve.wait_ge(s_cf, t + 1)
                        if t >= TC:
                            dve.wait_ge(s_act, conv_done(t))
                        sl = t % TC
                        tag_y[t] = tag(dve.tensor_tensor(
                            y16[t % 2][:], xh[:, sl * F:(sl + 1) * F],
                            cfull[t % N_CF][:],
                            op=mybir.AluOpType.mult))
                    for t in pair:
                        dve.wait_ge(s_dve, tag_y[t])
                        tag_pb[t] = tag(dve.tensor_scalar(
                            t16[t % 2][:].bitcast(i16),
                            y16[t % 2][:].bitcast(i16),
                            H_EXPMASK, None,
                            op0=mybir.AluOpType.bitwise_and))
                    for t in pair:
                        dve.wait_ge(s_dve, tag_pb[t])
                        tag_pb[t] = tag(dve.tensor_scalar(
                            pb16[t % 2][:], t16[t % 2][:].bitcast(i16),
                            H_MAGIC_ADD, H_MAGIC_MIN,
                            op0=mybir.AluOpType.add,
                            op1=mybir.AluOpType.max))
                    for t in pair:
                        dve.wait_ge(s_dve, tag_pb[t])
                        tag_t[t] = tag(dve.tensor_tensor(
                            t16[t % 2][:], y16[t % 2][:],
                            pb16[t % 2][:].bitcast(f16),
                            op=mybir.AluOpType.add))
                    for t in pair:
                        dve.wait_ge(s_dve, tag_t[t])
                        tag_nq[t] = tag(dve.tensor_tensor(
                            nq16[t % 2][:], pb16[t % 2][:].bitcast(f16),
                            t16[t % 2][:], op=mybir.AluOpType.subtract))
                    for t in pair:
                        bsl = slice(t * FBLK, (t + 1) * FBLK)
                        dve.wait_ge(s_dve, tag_nq[t])
                        if t >= 2:
                            dve.wait_ge(s_ob[t % N_OB],
                                        16 * ((t - 2) // 2 + 1))
                        if t in CH_GATE:
                            dve.wait_ge(s_dve, K_nic[CH_GATE[t]])
                        tag_o[t] = tag(dve.tensor_tensor(
                            b3(ob[t % N_OB][:]), b3(nq16[t % 2][:]),
                            nic16_t[:, bsl].unsqueeze(-1).broadcast_to(
                                [P, FBLK, 16]),
                            op=mybir.AluOpType.mult))

            @block.scalar
            def _(act):
                # pass A: odd input DMAs issue from this queue (doubles
                # HWDGE issue bandwidth), interleaved with the fp16
                # conversions of the cached tiles
                for t in range(T):
                    if t % 2 == 1 and t >= 3:
                        prev = XA_PREV[(t, False)]
                        if prev is not None:
                            pt = prev[0]
                            act.wait_ge(s_dve, dveA[pt])
                            if pt < TC:
                                act.wait_ge(s_act, conv_done(pt))
                        act.dma_start(
                            out=xa[t % N_XA][:, :],
                            in_=x_ext[:, t * F:(t + 1) * F],
                        ).then_inc(s_xa[t % N_XA], 16)
                    c = t - 1
                    if 0 <= c < TC:
                        act.wait_ge(s_xa[XA_NEED[(c, False)][0]],
                                    XA_NEED[(c, False)][1])
                        act.activation(
                            xh[:, c * F:(c + 1) * F], xa[c % N_XA][:],
                            mybir.ActivationFunctionType.Copy,
                        ).then_inc(s_act)
                # rm = 1/m, in place, per quarter (AR-independent; gated
                # on the quarter's partial max so m is fully consumed)
                for q in range(NQ):
                    act.wait_ge(s_dve, K_mxq[q])
                    act_reciprocal(act, m_t[:, qs(q)],
                                   m_t[:, qs(q)]).then_inc(s_acr)
                # rs = 1/s_b per chunk
                for c in range(NCH):
                    cs = slice(CH_START[c], CH_START[c] + CHUNKS[c])
                    act.wait_ge(s_dve, K_sb[c])
                    if c >= 2:
                        act.wait_ge(s_dve, K_nic[c - 2])
                    act_reciprocal(act, rs2[c % 2][:, 0:CHUNKS[c]],
                                   m_t[:, cs]).then_inc(s_acr)

                # pass B: cfull per tile; re-read conversions (into the
                # ring slot their predecessor vacated) run 3 tiles ahead
                # of their consumer so the DVE never waits on them
                def conv(t):
                    act.wait_ge(s_xa[XA_NEED[(t, True)][0]],
                                XA_NEED[(t, True)][1])
                    act.wait_ge(s_dve, tag_y[t - TC])
                    sl = t % TC
                    act.activation(
                        xh[:, sl * F:(sl + 1) * F], xa[t % N_XA][:],
                        mybir.ActivationFunctionType.Copy,
                    ).then_inc(s_act)

                for t in range(T):
                    bsl = slice(t * FBLK, (t + 1) * FBLK)
                    if t >= N_CF:
                        act.wait_ge(s_dve, tag_y[t - N_CF])
                    act.wait_ge(s_dve, K_c[chunk_of_tile(t)])
                    act.activation(
                        b3(cfull[t % N_CF][:]),
                        c16_t[:, bsl].unsqueeze(-1).broadcast_to(
                            [P, FBLK, 16]),
                        mybir.ActivationFunctionType.Copy,
                    ).then_inc(s_cf)
                    r = t + 3
                    if TC <= r < T:
                        conv(r)

            @block.gpsimd
            def _(pool):
                pool.memset(gall_t[0:1, :], 0.0).then_inc(s_pool)
                pool.wait_ge(s_pool, 1)
                pool.dma_start(out=cc_warm_in[:, :],
                               in_=gall_t[0:1, :]).then_inc(s_warm, 16)
                pool.wait_ge(s_warm, 16)
                pool.collective_compute(
                    "AllReduce",
                    mybir.AluOpType.max,
                    replica_groups=[list(range(n_cores))],
                    ins=[cc_warm_in.ap().opt()],
                    outs=[cc_warm_out.ap().opt()],
                ).then_inc(s_cc)
                pool.wait_ge(s_cdma, 16)
                pool.collective_compute(
                    "AllReduce",
                    mybir.AluOpType.max,
                    replica_groups=[list(range(n_cores))],
                    ins=[cc_in.ap().opt()],
                    outs=[cc_out.ap().opt()],
                ).then_inc(s_cc)
                # pass-B re-read DMAs (tiles TC+EARLY_RR..), issued from
                # this otherwise-idle queue as their xa slot frees
                for r in range(TC + EARLY_RR, T):
                    prev = XA_PREV[(r, True)]
                    pt = prev[0]
                    pool.wait_ge(s_act, conv_done(pt))
                    pool.dma_start(
                        out=xa[r % N_XA][:, :],
                        in_=x_ext[:, r * F:(r + 1) * F],
                    ).then_inc(s_xa[r % N_XA], 16)

            @block.sync
            def _(sync):
                def rr_wait(t):
                    prev = XA_PREV[(t, True)]
                    if prev is None:
                        return
                    pt, was_rr = prev
                    if not was_rr:
                        sync.wait_ge(s_dve, dveA[pt])
                        if pt < TC:
                            sync.wait_ge(s_act, conv_done(pt))
                    else:
                        sync.wait_ge(s_act, conv_done(pt))

                # pass A input DMAs (tiles 0,1,2 primed here, then even
                # tiles; odd tiles >=3 issue from the ACT queue)
                for t in [0, 1] + list(range(2, T, 2)):
                    prev = XA_PREV[(t, False)]
                    if prev is not None:
                        pt = prev[0]
                        sync.wait_ge(s_dve, dveA[pt])
                        if pt < TC:
                            sync.wait_ge(s_act, conv_done(pt))
                    sync.dma_start(
                        out=xa[t % N_XA][:, :],
                        in_=x_ext[:, t * F:(t + 1) * F],
                    ).then_inc(s_xa[t % N_XA], 16)
                # early re-read prefetches (overlap the AllReduce window)
                for t in range(TC, TC + EARLY_RR):
                    rr_wait(t)
                    sync.dma_start(
                        out=xa[t % N_XA][:, :],
                        in_=x_ext[:, t * F:(t + 1) * F],
                    ).then_inc(s_xa[t % N_XA], 16)
                # collective staging
                sync.wait_ge(s_dve, K_mx[0])
                sync.dma_start(out=cc_in[:, :], in_=mx_t[:, :]).then_inc(
                    s_cdma, 16)
                sync.wait_ge(s_cc, 2)
                sync.dma_start(
                    out=gall_t[:, :],
                    in_=cc_out.ap().broadcast_to([P, 128]),
                ).then_inc(s_cdma, 16)
                # pass B: out DMAs only (re-reads go via the idle GPSIMD
                # queue so this queue stays under the output bandwidth)
                for t in range(T):
                    sync.wait_ge(s_dve, tag_o[t])
                    sync.dma_start(
                        out=out_ext[:, t * F:(t + 1) * F],
                        in_=ob[t % N_OB][:, :],
                    ).then_inc(s_ob[t % N_OB], 16)
                for i in range(N_OB):
                    uses = len([t for t in range(T) if t % N_OB == i])
                    sync.wait_ge(s_ob[i], 16 * uses)

    return nc


_CACHE = {}


def _get_nc():
    if "nc" not in _CACHE:
        _CACHE["nc"] = build_nc()
    return _CACHE["nc"]


def kernel(x: np.ndarray) -> np.ndarray:
    from concourse.bass_utils import run_bass_kernel_spmd

    x = np.asarray(x, dtype=np.float32)
    assert x.shape == FULL_SHAPE
    shards = x.reshape(N_CORES, P, L)
    in_maps = [{"x": np.ascontiguousarray(shards[i])} for i in range(N_CORES)]
    nc = _get_nc()
    res = run_bass_kernel_spmd(nc, in_maps, core_ids=list(range(N_CORES)))
    out = np.stack([r["out"] for r in res.results], axis=0)
    return out.reshape(FULL_SHAPE)



# revision 5
# speedup vs baseline: 1.0631x; 1.0631x over previous
"""NVFP4-style activation quantizer on 8 TRN2 NeuronCores (raw bass).

Reference semantics (per 16-element block, fp32):
    s_t  = max|x| / (6*448)                      (global, needs all-reduce)
    m_b  = max|x| over block
    inv  = 6 / (m_b / s_t)
    s_b  = fp8_e4m3_roundtrip(inv)   (the 0/inf guard is dead code for this
                                      input: inv >= 6/2688 = 2.23e-3 > 2^-10)
    out  = sign(x) * fp4_121(|x|/s_t * s_b) / s_b * s_t

Core trick: a runtime-registered custom DVE op fuses the whole fp4_121
magic-round into ONE DVE instruction per tile (6 ALU stages, 1 elem/cyc):

    y = Src0 * Src1            (x fp16 * per-block c, stride-0 bcast in1)
    p = y & 0x7F800000         (exponent bits as float = 2^e(y); s0 = +inf)
    M = max(p, 1) * 6291456    (1.5*2^22 * max(2^e,1): ulp(M) = fp4 step)
    q = (y + M) - M            (RNE to the fp4_121 grid, sign handled)

fp32 internally, so no fp16 rounding of y (sim rel_l2 7.5e-3 vs the 2e-2
gate; measured on HW bit-exact vs the numpy model of this chain).

Structure: ONE read of x (full shard cached in SBUF as fp16, 16MB),
fp16 output (host upcasts), so HBM traffic is 32MB in + 16MB out/core.
Pass A: input DMAs (single SYNC queue) -> ACT fp32->fp16 convert into
the xh cache + DVE per-block abs-max -> quarter maxes -> AllReduce
(warmed up, on the GPSIMD queue).  Post-AR the scale chain runs
DVE (f8 roundtrip) + ACT (c16 = sb/st, rs = 1/sb, nic = st/sb) in 5
chunks (micro first chunk).  Pass B: custom op1 per tile on DVE; the
final per-block multiply o = q*nic runs on GPSIMD for ~2/3 of tiles
(Q7 TT multiply at 0.42 eff) and DVE for the rest, writing fp16 output
over the dead xh slot; out-DMAs trail on the SYNC queue.
"""

import numpy as np

FULL_SHAPE = (4, 4096, 4096)
N_CORES = 8
P = 128
TOTAL = 4 * 4096 * 4096
L = TOTAL // (N_CORES * P)   # 65536 elements per partition per core
NBLK = L // 16               # 4096 blocks per partition

F = 2048
T = L // F                   # 32 tiles
FBLK = F // 16               # 128 blocks per tile
NQ = 4
QBLK = NBLK // NQ
TQ = T // NQ                 # 8 tiles per quarter
N_XA = 3
N_Q = 3                      # q16 ring

MAGIC = 6291456.0            # 1.5 * 2^22

# scale-chain chunks (in blocks): micro first chunk -> eat AR latency
CHUNKS = [128, 896, 1024, 1024, 1024]
CH_START = [sum(CHUNKS[:i]) for i in range(len(CHUNKS))]
NCH = len(CHUNKS)
# chunk -> number of ACT rm-quarters that must be done first
RMQ = [1, 1, 2, 3, 4]


def chunk_of_tile(t):
    b = t * FBLK
    for c in range(NCH):
        if CH_START[c] <= b < CH_START[c] + CHUNKS[c]:
            return c
    raise ValueError(t)


def dve_owns_o(t):
    # DVE handles ~1/3 of the final multiplies, GPSIMD the rest
    return t % 3 == 2


def register_fp4_op():
    from concourse import dve_ops
    from concourse.dve_spec import (
        Spec, Src0, Src1, C0, C1, Bin, AluOp, maxx, One, lower, _has_src1,
    )
    from concourse.dve_uop import DveOpSpec

    NAME = "FP4_SCALE_ROUND_ANT"
    for o in dve_ops.OPS:
        if o.name == NAME:
            return o

    y = Src0 * Src1
    p = Bin(AluOp.BITWISE_AND, y, C0)
    M = maxx(p, One) * C1
    q = (y + M) - M

    def ref(in0, in1, s0, s1, imm2):
        yy = in0.astype(np.float32) * np.asarray(in1, np.float32)
        pp = (yy.view(np.int32) & np.int32(0x7F800000)).view(np.float32)
        MM = np.maximum(pp, 1.0) * np.float32(s1)
        tt = (yy + MM).astype(np.float32)
        return (tt - MM).astype(np.float32)

    spec = Spec(body=q, reference=ref)
    row = max(dve_ops._SUB_OPCODE_FOR_NAME.values()) + 1
    assert row < 0x20
    dve_ops._SUB_OPCODE_FOR_NAME[NAME] = row
    uops = lower(spec, ver="v3")
    sha = DveOpSpec(name=NAME, opcode=row, uops=uops,
                    rd1_en=_has_src1(spec)).sha("v3")
    op = dve_ops.DveOp(NAME, spec, subdim=False, uops_sha={"v3": sha})
    dve_ops.OPS.append(op)
    dve_ops.CUSTOM_DVE_SPECS[NAME] = spec
    return op


def build_nc(n_cores=N_CORES):
    from contextlib import ExitStack

    import concourse.bass as bass
    from concourse import mybir

    fp4_op = register_fp4_op()

    f32 = mybir.dt.float32
    f16 = mybir.dt.float16
    f8 = mybir.dt.float8e4

    nc = bass.Bass(num_devices=n_cores, debug=False)
    x_ext = nc.declare_dram_parameter("x", [P, L], f32, isOutput=False)
    out_ext = nc.declare_dram_parameter("out", [P, L], f16, isOutput=True)
    cc_in = nc.dram_tensor("cc_in", [1, 128], f32)
    cc_out = nc.dram_tensor("cc_out", [1, 128], f32, addr_space="Shared")
    cc_warm_in = nc.dram_tensor("cc_warm_in", [1, 128], f32)
    cc_warm_out = nc.dram_tensor("cc_warm_out", [1, 128], f32,
                                 addr_space="Shared")

    def act_reciprocal(act, out, in_):
        return act.add_instruction(
            mybir.InstActivation(
                name=act.bass.get_next_instruction_name(),
                func=mybir.ActivationFunctionType.Reciprocal,
                ins=[
                    act.lower_ap(in_),
                    mybir.ImmediateValue(dtype=f32, value=0.0),
                    mybir.ImmediateValue(dtype=f32, value=1.0),
                    mybir.ImmediateValue(dtype=f32, value=0.0),
                ],
                outs=[act.lower_ap(out)],
            )
        )

    with ExitStack() as ctx:
        def sem(name):
            return ctx.enter_context(nc.semaphore(name))

        def sbuf(name, shape, dt=f32):
            return ctx.enter_context(nc.sbuf_tensor(name, shape, dt))

        s_xa = [sem(f"s_xa{i}") for i in range(N_XA)]
        s_dve = sem("s_dve")
        s_conv = sem("s_conv")   # +1 per ACT conv (count = t+1)
        s_rm = sem("s_rm")       # +1 per ACT rm quarter (count = q+1)
        s_c16 = sem("s_c16")     # +1 per ACT c16 chunk (count = c+1)
        s_nic = sem("s_nic")     # +1 per ACT nic chunk (count = c+1)
        s_rs = sem("s_rs")       # +1 per ACT rs chunk (count = c+1)
        s_pool = sem("s_pool")   # +1 per pool o-instruction
        s_cdma = sem("s_cdma")
        s_cc = sem("s_cc")
        s_warm = sem("s_warm")
        s_out = sem("s_out")
        s_ps = sem("s_ps")       # pool memset staging

        xh = sbuf("xh", [P, T * F], f16)            # 16MB: cache + output
        xa = [sbuf(f"xa{i}", [P, F]) for i in range(N_XA)]
        q16 = [sbuf(f"q16_{i}", [P, F], f16) for i in range(N_Q)]
        m_t = sbuf("m_t", [P, NBLK])                # blockmax -> 1/m -> s_b
        rs2 = [sbuf(f"rs2_{i}", [P, 1024]) for i in range(2)]
        f8_t = sbuf("f8_t", [P, 1024], f8)
        c16_t = sbuf("c16_t", [P, NBLK], f16)
        nic16_t = sbuf("nic16_t", [P, NBLK], f16)
        gall_t = sbuf("gall_t", [P, 128])
        mxq_t = sbuf("mxq_t", [P, NQ])
        mx_t = sbuf("mx_t", [P, 1])
        g128_t = sbuf("g128_t", [P, 1])
        st_t = sbuf("st_t", [P, 1])
        rt_t = sbuf("rt_t", [P, 1])
        k6_t = sbuf("k6_t", [P, 1])
        emask_t = sbuf("emask_t", [P, 1])

        dveA = [0] * T            # s_dve after reduce(t)
        tag_o_pool = [0] * T      # s_pool after o(t), static
        _pc = 0
        for _t in range(T):
            if not dve_owns_o(_t):
                _pc += 1
                tag_o_pool[_t] = _pc
        K_mxq = [0] * NQ
        K_mx = [0]
        K_sb = [0] * NCH
        tag_q = [0] * T           # s_dve after op1(t)
        tag_o_dve = [0] * T       # s_dve after o(t) (dve-owned)

        def b3(ap):
            return ap.rearrange("p (b s) -> p b s", s=16)

        def qs(q):
            return slice(q * QBLK, (q + 1) * QBLK)

        def xs(t):
            return slice(t * F, (t + 1) * F)

        def cslice(c):
            return slice(CH_START[c], CH_START[c] + CHUNKS[c])

        def bcast(tens, t):
            bsl = slice(t * FBLK, (t + 1) * FBLK)
            return tens[:, bsl].unsqueeze(-1).broadcast_to([P, FBLK, 16])

        with nc.Block() as block:

            @block.vector
            def _(dve):
                cnt = 0

                def tag(ins):
                    nonlocal cnt
                    ins.then_inc(s_dve)
                    cnt += 1
                    return cnt

                tag(dve.memset(emask_t[:], float("inf")))

                # ---- pass A: per-block abs max (fp32, from xa ring) ----
                for t in range(T):
                    dve.wait_ge(s_xa[t % N_XA], 16 * (t // N_XA + 1))
                    dveA[t] = tag(dve.tensor_reduce(
                        out=m_t[:, t * FBLK:(t + 1) * FBLK],
                        in_=b3(xa[t % N_XA][:]),
                        axis=mybir.AxisListType.X,
                        op=mybir.AluOpType.max,
                        apply_absolute_value=True,
                    ))
                    if (t + 1) % TQ == 0:
                        q = t // TQ
                        dve.wait_ge(s_dve, dveA[t])
                        K_mxq[q] = tag(dve.tensor_reduce(
                            out=mxq_t[:, q:q + 1], in_=m_t[:, qs(q)],
                            axis=mybir.AxisListType.X,
                            op=mybir.AluOpType.max,
                        ))
                dve.wait_ge(s_dve, K_mxq[NQ - 1])
                K_mx[0] = tag(dve.tensor_reduce(
                    out=mx_t[:], in_=mxq_t[:], axis=mybir.AxisListType.X,
                    op=mybir.AluOpType.max,
                ))

                # ---- global scalars (post-AllReduce) ----
                dve.wait_ge(s_cdma, 32)
                k_g = tag(dve.tensor_reduce(
                    out=g128_t[:], in_=gall_t[:], axis=mybir.AxisListType.X,
                    op=mybir.AluOpType.max))
                dve.wait_ge(s_dve, k_g)
                k_st = tag(dve.tensor_scalar(
                    st_t[:], g128_t[:], 1.0 / 2688.0, None,
                    op0=mybir.AluOpType.mult))
                dve.wait_ge(s_dve, k_st)
                tag(dve.tensor_scalar(
                    k6_t[:], st_t[:], 6.0, None, op0=mybir.AluOpType.mult))
                k_rt = tag(dve.reciprocal(rt_t[:], st_t[:]))

                # ---- per-block scale chain: f8 roundtrip per chunk ----
                # (rm = 1/m already in m_t from ACT, per quarter)
                def emit_chunk(c):
                    cs = cslice(c)
                    n = CHUNKS[c]
                    dve.wait_ge(s_rm, RMQ[c])
                    dve.wait_ge(s_dve, k_rt if c == 0 else K_sb[c - 1])
                    k_f8 = tag(dve.tensor_scalar(
                        f8_t[:, 0:n], m_t[:, cs], k6_t[:], None,
                        op0=mybir.AluOpType.mult))
                    dve.wait_ge(s_dve, k_f8)
                    K_sb[c] = tag(dve.tensor_copy(m_t[:, cs], f8_t[:, 0:n]))

                emit_chunk(0)
                emit_chunk(1)

                # ---- pass B ----
                next_chunk = 2
                for t in range(T):
                    # keep the scale chain 1 chunk ahead of the op1 stream
                    while (next_chunk < NCH
                           and t >= (CH_START[next_chunk] // FBLK) - 7):
                        emit_chunk(next_chunk)
                        next_chunk += 1
                    if t >= N_Q:
                        tp = t - N_Q
                        if not dve_owns_o(tp):
                            dve.wait_ge(s_pool, tag_o_pool[tp])
                        else:
                            dve.wait_ge(s_dve, tag_o_dve[tp])
                    dve.wait_ge(s_c16, chunk_of_tile(t) + 1)
                    tag_q[t] = tag(dve._custom_dve(
                        fp4_op,
                        out=b3(q16[t % N_Q][:]),
                        in0=b3(xh[:, xs(t)]),
                        in1=bcast(c16_t, t),
                        s0=emask_t[:],
                        s1=MAGIC,
                    ))
                    if dve_owns_o(t):
                        dve.wait_ge(s_nic, chunk_of_tile(t) + 1)
                        dve.wait_ge(s_dve, tag_q[t])
                        tag_o_dve[t] = tag(dve.tensor_tensor(
                            b3(xh[:, xs(t)]), b3(q16[t % N_Q][:]),
                            bcast(nic16_t, t),
                            op=mybir.AluOpType.mult))

            @block.scalar
            def _(act):
                # pass A: fp32 -> fp16 conversions into the cache;
                # rm = 1/m per quarter as its partial max completes
                for t in range(T):
                    act.wait_ge(s_xa[t % N_XA], 16 * (t // N_XA + 1))
                    act.activation(
                        xh[:, xs(t)], xa[t % N_XA][:],
                        mybir.ActivationFunctionType.Copy,
                    ).then_inc(s_conv)
                    if (t + 1) % TQ == 0:
                        q = t // TQ
                        act.wait_ge(s_dve, K_mxq[q])
                        act_reciprocal(
                            act, m_t[:, qs(q)], m_t[:, qs(q)]).then_inc(s_rm)

                # post-AR scale chain per chunk: c16 = rt*sb -> fp16,
                # rs = 1/sb, nic = st*rs -> fp16
                for c in range(NCH):
                    cs = cslice(c)
                    n = CHUNKS[c]
                    act.wait_ge(s_dve, K_sb[c])
                    act.activation(
                        c16_t[:, cs], m_t[:, cs],
                        mybir.ActivationFunctionType.Copy,
                        scale=rt_t[:],
                    ).then_inc(s_c16)
                    act_reciprocal(
                        act, rs2[c % 2][:, 0:n], m_t[:, cs]).then_inc(s_rs)
                    act.wait_ge(s_rs, c + 1)
                    act.activation(
                        nic16_t[:, cs], rs2[c % 2][:, 0:n],
                        mybir.ActivationFunctionType.Copy,
                        scale=st_t[:],
                    ).then_inc(s_nic)

            @block.gpsimd
            def _(pool):
                pool.memset(gall_t[0:1, :], 0.0).then_inc(s_ps)
                pool.wait_ge(s_ps, 1)
                pool.dma_start(out=cc_warm_in[:, :],
                               in_=gall_t[0:1, :]).then_inc(s_warm, 16)
                pool.wait_ge(s_warm, 16)
                pool.collective_compute(
                    "AllReduce",
                    mybir.AluOpType.max,
                    replica_groups=[list(range(n_cores))],
                    ins=[cc_warm_in.ap().opt()],
                    outs=[cc_warm_out.ap().opt()],
                ).then_inc(s_cc)
                pool.wait_ge(s_cdma, 16)
                pool.collective_compute(
                    "AllReduce",
                    mybir.AluOpType.max,
                    replica_groups=[list(range(n_cores))],
                    ins=[cc_in.ap().opt()],
                    outs=[cc_out.ap().opt()],
                ).then_inc(s_cc)
                # final multiplies for pool-owned tiles
                for t in range(T):
                    if dve_owns_o(t):
                        continue
                    pool.wait_ge(s_dve, tag_q[t])
                    pool.wait_ge(s_nic, chunk_of_tile(t) + 1)
                    pool.tensor_tensor(
                        b3(xh[:, xs(t)]), b3(q16[t % N_Q][:]),
                        bcast(nic16_t, t),
                        op=mybir.AluOpType.mult).then_inc(s_pool)

            @block.sync
            def _(sync):
                # pass A input DMAs (single queue; slot gated on consumers)
                for t in range(T):
                    if t >= N_XA:
                        sync.wait_ge(s_dve, dveA[t - N_XA])
                        sync.wait_ge(s_conv, t - N_XA + 1)
                    sync.dma_start(
                        out=xa[t % N_XA][:, :],
                        in_=x_ext[:, xs(t)],
                    ).then_inc(s_xa[t % N_XA], 16)
                # collective staging
                sync.wait_ge(s_dve, K_mx[0])
                sync.dma_start(out=cc_in[:, :], in_=mx_t[:, :]).then_inc(
                    s_cdma, 16)
                sync.wait_ge(s_cc, 2)
                sync.dma_start(
                    out=gall_t[:, :],
                    in_=cc_out.ap().broadcast_to([P, 128]),
                ).then_inc(s_cdma, 16)
                # pass B: output DMAs (fp16, from the dead xh slot)
                for t in range(T):
                    if dve_owns_o(t):
                        sync.wait_ge(s_dve, tag_o_dve[t])
                    else:
                        sync.wait_ge(s_pool, tag_o_pool[t])
                    sync.dma_start(
                        out=out_ext[:, xs(t)],
                        in_=xh[:, xs(t)],
                    ).then_inc(s_out, 16)
                sync.wait_ge(s_out, 16 * T)

    mybir.codegen_inst_isa_subclasses(nc)
    return nc


_CACHE = {}


def _get_nc():
    if "nc" not in _CACHE:
        _CACHE["nc"] = build_nc()
    return _CACHE["nc"]


def kernel(x: np.ndarray) -> np.ndarray:
    from concourse.bass_utils import run_bass_kernel_spmd

    x = np.asarray(x, dtype=np.float32)
    assert x.shape == FULL_SHAPE
    shards = x.reshape(N_CORES, P, L)
    in_maps = [{"x": np.ascontiguousarray(shards[i])} for i in range(N_CORES)]
    nc = _get_nc()
    res = run_bass_kernel_spmd(nc, in_maps, core_ids=list(range(N_CORES)))
    out = np.stack([np.asarray(r["out"], dtype=np.float32)
                    for r in res.results], axis=0)
    return out.reshape(FULL_SHAPE)


# revision 9
# speedup vs baseline: 1.2251x; 1.1524x over previous
"""NVFP4-style activation quantizer on 8 TRN2 NeuronCores (raw bass).

Reference semantics (per 16-element block, fp32):
    s_t  = max|x| / (6*448)                      (global, needs all-reduce)
    m_b  = max|x| over block
    inv  = 6 / (m_b / s_t)
    s_b  = fp8_e4m3_roundtrip(inv)   (the 0/inf guard is dead code for this
                                      input: inv >= 6/2688 = 2.23e-3 > 2^-10)
    out  = sign(x) * fp4_121(|x|/s_t * s_b) / s_b * s_t

Core trick: a runtime-registered custom DVE op fuses the whole fp4_121
magic-round into ONE DVE instruction per tile (6 ALU stages, 1 elem/cyc):

    y = Src0 * Src1            (x fp16 * per-block c, stride-0 bcast in1)
    p = y & 0x7F800000         (exponent bits as float = 2^e(y); s0 = +inf)
    M = max(p, 1) * 6291456    (1.5*2^22 * max(2^e,1): ulp(M) = fp4 step)
    q = (y + M) - M            (RNE to the fp4_121 grid, sign handled)

fp32 internally, so no fp16 rounding of y (sim rel_l2 7.5e-3 vs the 2e-2
gate; measured on HW bit-exact vs the numpy model of this chain).

Structure: ONE read of x (full shard cached in SBUF as fp16, 16MB),
fp16 output (host upcasts), so HBM traffic is 32MB in + 16MB out/core.
Pass A: input DMAs (single SYNC queue) -> ACT fp32->fp16 convert into
the xh cache + DVE per-block abs-max -> quarter maxes -> AllReduce
(warmed up, on the GPSIMD queue).  Post-AR the scale chain runs
DVE (f8 roundtrip) + ACT (c16 = sb/st, rs = 1/sb, nic = st/sb) in 5
chunks (micro first chunk).  Pass B: custom op1 per tile on DVE; the
final per-block multiply o = q*nic runs on GPSIMD for ~2/3 of tiles
(Q7 TT multiply at 0.42 eff) and DVE for the rest, writing fp16 output
over the dead xh slot; out-DMAs trail on the SYNC queue.
"""

import numpy as np

FULL_SHAPE = (4, 4096, 4096)
N_CORES = 8
P = 128
TOTAL = 4 * 4096 * 4096
L = TOTAL // (N_CORES * P)   # 65536 elements per partition per core
NBLK = L // 16               # 4096 blocks per partition

F = 2048
T = L // F                   # 32 tiles
FBLK = F // 16               # 128 blocks per tile
NQ = 4
QBLK = NBLK // NQ
TQ = T // NQ                 # 8 tiles per quarter
N_XA = 3
N_Q = 2                      # q16 ring
N_NF = 2                     # nicfull ring

MAGIC = 6291456.0            # 1.5 * 2^22

# scale-chain chunks (in blocks): micro first chunk -> eat AR latency
CHUNKS = [128, 896, 1024, 1024, 1024]
CH_START = [sum(CHUNKS[:i]) for i in range(len(CHUNKS))]
NCH = len(CHUNKS)
# chunk -> number of ACT rm-quarters that must be done first
RMQ = [1, 1, 2, 3, 4]


def chunk_of_tile(t):
    b = t * FBLK
    for c in range(NCH):
        if CH_START[c] <= b < CH_START[c] + CHUNKS[c]:
            return c
    raise ValueError(t)


def register_fp4_op():
    from concourse import dve_ops
    from concourse.dve_spec import (
        Spec, Src0, Src1, C0, C1, Bin, AluOp, maxx, One, lower, _has_src1,
    )
    from concourse.dve_uop import DveOpSpec

    NAME = "FP4_SCALE_ROUND_ANT"
    for o in dve_ops.OPS:
        if o.name == NAME:
            return o

    y = Src0 * Src1
    p = Bin(AluOp.BITWISE_AND, y, C0)
    M = maxx(p, One) * C1
    q = (y + M) - M

    def ref(in0, in1, s0, s1, imm2):
        yy = in0.astype(np.float32) * np.asarray(in1, np.float32)
        pp = (yy.view(np.int32) & np.int32(0x7F800000)).view(np.float32)
        MM = np.maximum(pp, 1.0) * np.float32(s1)
        tt = (yy + MM).astype(np.float32)
        return (tt - MM).astype(np.float32)

    spec = Spec(body=q, reference=ref)
    row = max(dve_ops._SUB_OPCODE_FOR_NAME.values()) + 1
    assert row < 0x20
    dve_ops._SUB_OPCODE_FOR_NAME[NAME] = row
    uops = lower(spec, ver="v3")
    sha = DveOpSpec(name=NAME, opcode=row, uops=uops,
                    rd1_en=_has_src1(spec)).sha("v3")
    op = dve_ops.DveOp(NAME, spec, subdim=False, uops_sha={"v3": sha})
    dve_ops.OPS.append(op)
    dve_ops.CUSTOM_DVE_SPECS[NAME] = spec
    return op


def build_nc(n_cores=N_CORES):
    from contextlib import ExitStack

    import concourse.bass as bass
    from concourse import mybir

    fp4_op = register_fp4_op()

    f32 = mybir.dt.float32
    f16 = mybir.dt.float16
    f8 = mybir.dt.float8e4

    nc = bass.Bass(num_devices=n_cores, debug=False)
    x_ext = nc.declare_dram_parameter("x", [P, L], f32, isOutput=False)
    out_ext = nc.declare_dram_parameter("out", [P, L], f16, isOutput=True)
    cc_in = nc.dram_tensor("cc_in", [1, 128], f32)
    cc_out = nc.dram_tensor("cc_out", [1, 128], f32, addr_space="Shared")
    cc_warm_in = nc.dram_tensor("cc_warm_in", [1, 128], f32)
    cc_warm_out = nc.dram_tensor("cc_warm_out", [1, 128], f32,
                                 addr_space="Shared")

    def act_reciprocal(act, out, in_):
        return act.add_instruction(
            mybir.InstActivation(
                name=act.bass.get_next_instruction_name(),
                func=mybir.ActivationFunctionType.Reciprocal,
                ins=[
                    act.lower_ap(in_),
                    mybir.ImmediateValue(dtype=f32, value=0.0),
                    mybir.ImmediateValue(dtype=f32, value=1.0),
                    mybir.ImmediateValue(dtype=f32, value=0.0),
                ],
                outs=[act.lower_ap(out)],
            )
        )

    with ExitStack() as ctx:
        def sem(name):
            return ctx.enter_context(nc.semaphore(name))

        def sbuf(name, shape, dt=f32):
            return ctx.enter_context(nc.sbuf_tensor(name, shape, dt))

        s_xa = [sem(f"s_xa{i}") for i in range(N_XA)]
        s_dve = sem("s_dve")
        s_conv = sem("s_conv")   # +1 per ACT conv (count = t+1)
        s_rm = sem("s_rm")       # +1 per ACT rm quarter (count = q+1)
        s_c16 = sem("s_c16")     # +1 per ACT c16 chunk (count = c+1)
        s_rs = sem("s_rs")       # +1 per ACT rs chunk (count = c+1)
        s_nf = sem("s_nf")       # +1 per ACT nicfull tile (count = t+1)
        s_cdma = sem("s_cdma")
        s_cc = sem("s_cc")
        s_warm = sem("s_warm")
        s_out = sem("s_out")
        s_ps = sem("s_ps")       # pool memset staging

        xh = sbuf("xh", [P, T * F], f16)            # 16MB: cache + output
        xa = [sbuf(f"xa{i}", [P, F]) for i in range(N_XA)]
        q16 = [sbuf(f"q16_{i}", [P, F], f16) for i in range(N_Q)]
        nf16 = [sbuf(f"nf16_{i}", [P, F], f16) for i in range(N_NF)]
        m_t = sbuf("m_t", [P, NBLK])                # blockmax -> 1/m -> s_b
        rs2 = [sbuf(f"rs2_{i}", [P, 1024]) for i in range(2)]
        f8_t = sbuf("f8_t", [P, 1024], f8)
        c16_t = sbuf("c16_t", [P, NBLK], f16)
        gall_t = sbuf("gall_t", [P, 128])
        mxq_t = sbuf("mxq_t", [P, NQ])
        mx_t = sbuf("mx_t", [P, 1])
        g128_t = sbuf("g128_t", [P, 1])
        st_t = sbuf("st_t", [P, 1])
        rt_t = sbuf("rt_t", [P, 1])
        k6_t = sbuf("k6_t", [P, 1])
        emask_t = sbuf("emask_t", [P, 1])

        dveA = [0] * T            # s_dve after reduce(t)
        K_mxq = [0] * NQ
        K_mx = [0]
        K_sb = [0] * NCH
        tag_q = [0] * T           # s_dve after op1(t)
        tag_o_dve = [0] * T       # s_dve after o(t) (dve-owned)

        def b3(ap):
            return ap.rearrange("p (b s) -> p b s", s=16)

        def qs(q):
            return slice(q * QBLK, (q + 1) * QBLK)

        def xs(t):
            return slice(t * F, (t + 1) * F)

        def cslice(c):
            return slice(CH_START[c], CH_START[c] + CHUNKS[c])

        def bcast(tens, t):
            bsl = slice(t * FBLK, (t + 1) * FBLK)
            return tens[:, bsl].unsqueeze(-1).broadcast_to([P, FBLK, 16])

        with nc.Block() as block:

            @block.vector
            def _(dve):
                cnt = 0

                def tag(ins):
                    nonlocal cnt
                    ins.then_inc(s_dve)
                    cnt += 1
                    return cnt

                k_ms = tag(dve.memset(emask_t[:], float("inf")))
                # custom-op warmup: first use pays a large one-time cost;
                # run a tiny dummy now so it lands off the critical path
                dve.wait_ge(s_dve, k_ms)
                tag(dve._custom_dve(
                    fp4_op,
                    out=q16[0][:, 0:16].rearrange("p (b s) -> p b s", s=16),
                    in0=xh[:, 0:16].rearrange("p (b s) -> p b s", s=16),
                    in1=c16_t[:, 0:1].unsqueeze(-1).broadcast_to([P, 1, 16]),
                    s0=emask_t[:],
                    s1=MAGIC,
                ))

                # ---- pass A: per-block abs max (fp16, from the xh cache;
                # keeps the xa ring gated only on the ACT convert) ----
                for t in range(T):
                    dve.wait_ge(s_conv, t + 1)
                    dveA[t] = tag(dve.tensor_reduce(
                        out=m_t[:, t * FBLK:(t + 1) * FBLK],
                        in_=b3(xh[:, xs(t)]),
                        axis=mybir.AxisListType.X,
                        op=mybir.AluOpType.max,
                        apply_absolute_value=True,
                    ))
                    if (t + 1) % TQ == 0:
                        q = t // TQ
                        dve.wait_ge(s_dve, dveA[t])
                        K_mxq[q] = tag(dve.tensor_reduce(
                            out=mxq_t[:, q:q + 1], in_=m_t[:, qs(q)],
                            axis=mybir.AxisListType.X,
                            op=mybir.AluOpType.max,
                        ))
                dve.wait_ge(s_dve, K_mxq[NQ - 1])
                K_mx[0] = tag(dve.tensor_reduce(
                    out=mx_t[:], in_=mxq_t[:], axis=mybir.AxisListType.X,
                    op=mybir.AluOpType.max,
                ))

                # ---- global scalars (post-AllReduce) ----
                dve.wait_ge(s_cdma, 32)
                k_g = tag(dve.tensor_reduce(
                    out=g128_t[:], in_=gall_t[:], axis=mybir.AxisListType.X,
                    op=mybir.AluOpType.max))
                dve.wait_ge(s_dve, k_g)
                k_st = tag(dve.tensor_scalar(
                    st_t[:], g128_t[:], 1.0 / 2688.0, None,
                    op0=mybir.AluOpType.mult))
                dve.wait_ge(s_dve, k_st)
                tag(dve.tensor_scalar(
                    k6_t[:], st_t[:], 6.0, None, op0=mybir.AluOpType.mult))
                k_rt = tag(dve.reciprocal(rt_t[:], st_t[:]))

                # ---- per-block scale chain: f8 roundtrip per chunk ----
                # (rm = 1/m already in m_t from ACT, per quarter)
                def emit_chunk(c):
                    cs = cslice(c)
                    n = CHUNKS[c]
                    dve.wait_ge(s_rm, RMQ[c])
                    dve.wait_ge(s_dve, k_rt if c == 0 else K_sb[c - 1])
                    k_f8 = tag(dve.tensor_scalar(
                        f8_t[:, 0:n], m_t[:, cs], k6_t[:], None,
                        op0=mybir.AluOpType.mult))
                    dve.wait_ge(s_dve, k_f8)
                    K_sb[c] = tag(dve.tensor_copy(m_t[:, cs], f8_t[:, 0:n]))

                emit_chunk(0)
                emit_chunk(1)

                # ---- pass B: op1 (custom, bcast in1) + o (dense, 2x) ----
                next_chunk = 2
                for t in range(T):
                    # early chunks: all f8/sb done by tile 5
                    while (next_chunk < NCH
                           and t >= 2 * (next_chunk - 2) + 1):
                        emit_chunk(next_chunk)
                        next_chunk += 1
                    dve.wait_ge(s_c16, chunk_of_tile(t) + 1)
                    tag_q[t] = tag(dve._custom_dve(
                        fp4_op,
                        out=b3(q16[t % N_Q][:]),
                        in0=b3(xh[:, xs(t)]),
                        in1=bcast(c16_t, t),
                        s0=emask_t[:],
                        s1=MAGIC,
                    ))
                    dve.wait_ge(s_nf, t + 1)
                    dve.wait_ge(s_dve, tag_q[t])
                    tag_o_dve[t] = tag(dve.tensor_tensor(
                        xh[:, xs(t)], q16[t % N_Q][:], nf16[t % N_NF][:],
                        op=mybir.AluOpType.mult))

            @block.scalar
            def _(act):
                # pass A: fp32 -> fp16 conversions into the cache;
                # rm = 1/m per quarter as its partial max completes
                for t in range(T):
                    act.wait_ge(s_xa[t % N_XA], 16 * (t // N_XA + 1))
                    act.activation(
                        xh[:, xs(t)], xa[t % N_XA][:],
                        mybir.ActivationFunctionType.Copy,
                    ).then_inc(s_conv)
                    if (t + 1) % TQ == 0:
                        q = t // TQ
                        act.wait_ge(s_dve, K_mxq[q])
                        act_reciprocal(
                            act, m_t[:, qs(q)], m_t[:, qs(q)]).then_inc(s_rm)

                # post-AR scale chain per chunk: c16 = rt*sb -> fp16,
                # rs = 1/sb; then nicfull per tile straight from the rs
                # chunk (dense bcast materialization, scaled by st, so the
                # o-multiply runs 2x)
                def chain(c):
                    cs = cslice(c)
                    n = CHUNKS[c]
                    act.wait_ge(s_dve, K_sb[c])
                    act.activation(
                        c16_t[:, cs], m_t[:, cs],
                        mybir.ActivationFunctionType.Copy,
                        scale=rt_t[:],
                    ).then_inc(s_c16)
                    act_reciprocal(
                        act, rs2[c % 2][:, 0:n], m_t[:, cs]).then_inc(s_rs)

                def nf(t):
                    c = chunk_of_tile(t)
                    lo = t * FBLK - CH_START[c]
                    src_ap = rs2[c % 2][:, lo:lo + FBLK]
                    if t >= N_NF:
                        act.wait_ge(s_dve, tag_o_dve[t - N_NF])
                    act.wait_ge(s_rs, c + 1)
                    act.activation(
                        b3(nf16[t % N_NF][:]),
                        src_ap.unsqueeze(-1).broadcast_to([P, FBLK, 16]),
                        mybir.ActivationFunctionType.Copy,
                        scale=st_t[:],
                    ).then_inc(s_nf)

                chain(0)
                chain(1)
                nf(0)
                chain(2)
                for t in range(1, 8):
                    nf(t)
                chain(3)
                for t in range(8, 16):
                    nf(t)
                chain(4)
                for t in range(16, T):
                    nf(t)

            @block.gpsimd
            def _(pool):
                pool.memset(gall_t[0:1, :], 0.0).then_inc(s_ps)
                pool.wait_ge(s_ps, 1)
                pool.dma_start(out=cc_warm_in[:, :],
                               in_=gall_t[0:1, :]).then_inc(s_warm, 16)
                pool.wait_ge(s_warm, 16)
                pool.collective_compute(
                    "AllReduce",
                    mybir.AluOpType.max,
                    replica_groups=[list(range(n_cores))],
                    ins=[cc_warm_in.ap().opt()],
                    outs=[cc_warm_out.ap().opt()],
                ).then_inc(s_cc)
                pool.wait_ge(s_cdma, 16)
                pool.collective_compute(
                    "AllReduce",
                    mybir.AluOpType.max,
                    replica_groups=[list(range(n_cores))],
                    ins=[cc_in.ap().opt()],
                    outs=[cc_out.ap().opt()],
                ).then_inc(s_cc)

            @block.sync
            def _(sync):
                # pass A input DMAs (single queue; slot gated on consumers)
                for t in range(T):
                    if t >= N_XA:
                        sync.wait_ge(s_conv, t - N_XA + 1)
                    sync.dma_start(
                        out=xa[t % N_XA][:, :],
                        in_=x_ext[:, xs(t)],
                    ).then_inc(s_xa[t % N_XA], 16)
                # collective staging
                sync.wait_ge(s_dve, K_mx[0])
                sync.dma_start(out=cc_in[:, :], in_=mx_t[:, :]).then_inc(
                    s_cdma, 16)
                sync.wait_ge(s_cc, 2)
                sync.dma_start(
                    out=gall_t[:, :],
                    in_=cc_out.ap().broadcast_to([P, 128]),
                ).then_inc(s_cdma, 16)
                # pass B: output DMAs (fp16, from the dead xh slot)
                for t in range(T):
                    sync.wait_ge(s_dve, tag_o_dve[t])
                    sync.dma_start(
                        out=out_ext[:, xs(t)],
                        in_=xh[:, xs(t)],
                    ).then_inc(s_out, 16)
                sync.wait_ge(s_out, 16 * T)

    mybir.codegen_inst_isa_subclasses(nc)
    return nc


_CACHE = {}


def _get_nc():
    if "nc" not in _CACHE:
        _CACHE["nc"] = build_nc()
    return _CACHE["nc"]


def kernel(x: np.ndarray) -> np.ndarray:
    from concourse.bass_utils import run_bass_kernel_spmd

    x = np.asarray(x, dtype=np.float32)
    assert x.shape == FULL_SHAPE
    shards = x.reshape(N_CORES, P, L)
    in_maps = [{"x": np.ascontiguousarray(shards[i])} for i in range(N_CORES)]
    nc = _get_nc()
    res = run_bass_kernel_spmd(nc, in_maps, core_ids=list(range(N_CORES)))
    out = np.stack([np.asarray(r["out"], dtype=np.float32)
                    for r in res.results], axis=0)
    return out.reshape(FULL_SHAPE)


# revision 11
# speedup vs baseline: 1.4617x; 1.1931x over previous
"""NVFP4-style activation quantizer on 8 TRN2 NeuronCores (raw bass).

Reference semantics (per 16-element block, fp32):
    s_t  = max|x| / (6*448)                      (global, needs all-reduce)
    m_b  = max|x| over block
    inv  = 6 / (m_b / s_t)
    s_b  = fp8_e4m3_roundtrip(inv)   (the 0/inf guard is dead code for this
                                      input: inv >= 6/2688 = 2.23e-3 > 2^-10)
    out  = sign(x) * fp4_121(|x|/s_t * s_b) / s_b * s_t

Core trick: a runtime-registered custom DVE op fuses the whole fp4_121
magic-round into ONE DVE instruction per tile (6 ALU stages, 1 elem/cyc):

    y = Src0 * Src1            (x fp16 * per-block c, stride-0 bcast in1)
    p = y & 0x7F800000         (exponent bits as float = 2^e(y); s0 = +inf)
    M = max(p, 1) * 6291456    (1.5*2^22 * max(2^e,1): ulp(M) = fp4 step)
    q = (y + M) - M            (RNE to the fp4_121 grid, sign handled)

fp32 internally, so no fp16 rounding of y (sim rel_l2 7.5e-3 vs the 2e-2
gate; measured on HW bit-exact vs the numpy model of this chain).

Structure: ONE read of x (full shard cached in SBUF as fp16, 16MB),
fp16 output (host upcasts), so HBM traffic is 32MB in + 16MB out/core.
Pass A: input DMAs (single SYNC queue) -> ACT fp32->fp16 convert into
the xh cache + DVE per-block abs-max -> quarter maxes -> AllReduce
(warmed up, on the GPSIMD queue).  Post-AR the scale chain runs
DVE (f8 roundtrip) + ACT (c16 = sb/st, rs = 1/sb, nic = st/sb) in 5
chunks (micro first chunk).  Pass B: custom op1 per tile on DVE; the
final per-block multiply o = q*nic runs on GPSIMD for ~2/3 of tiles
(Q7 TT multiply at 0.42 eff) and DVE for the rest, writing fp16 output
over the dead xh slot; out-DMAs trail on the SYNC queue.
"""

import numpy as np

FULL_SHAPE = (4, 4096, 4096)
N_CORES = 8
P = 128
TOTAL = 4 * 4096 * 4096
L = TOTAL // (N_CORES * P)   # 65536 elements per partition per core
NBLK = L // 16               # 4096 blocks per partition

F = 2048
T = L // F                   # 32 tiles
FBLK = F // 16               # 128 blocks per tile
NQ = 4
QBLK = NBLK // NQ
TQ = T // NQ                 # 8 tiles per quarter
N_XA = 4
N_Q = 2                      # q16 ring
N_NF = 3                     # nicfull ring

MAGIC = 6291456.0            # 1.5 * 2^22

# scale-chain chunks (in blocks): micro first chunk -> eat AR latency
CHUNKS = [128, 896, 1024, 1024, 1024]
CH_START = [sum(CHUNKS[:i]) for i in range(len(CHUNKS))]
NCH = len(CHUNKS)
# chunk -> number of ACT rm-quarters that must be done first
RMQ = [1, 1, 2, 3, 4]


def chunk_of_tile(t):
    b = t * FBLK
    for c in range(NCH):
        if CH_START[c] <= b < CH_START[c] + CHUNKS[c]:
            return c
    raise ValueError(t)


def register_fp4_op():
    from concourse import dve_ops
    from concourse.dve_spec import (
        Spec, Src0, Src1, C0, C1, Bin, AluOp, maxx, One, lower, _has_src1,
    )
    from concourse.dve_uop import DveOpSpec

    NAME = "FP4_SCALE_ROUND_ANT"
    for o in dve_ops.OPS:
        if o.name == NAME:
            return o

    y = Src0 * Src1
    p = Bin(AluOp.BITWISE_AND, y, C0)
    M = maxx(p, One) * C1
    q = (y + M) - M

    def ref(in0, in1, s0, s1, imm2):
        yy = in0.astype(np.float32) * np.asarray(in1, np.float32)
        pp = (yy.view(np.int32) & np.int32(0x7F800000)).view(np.float32)
        MM = np.maximum(pp, 1.0) * np.float32(s1)
        tt = (yy + MM).astype(np.float32)
        return (tt - MM).astype(np.float32)

    spec = Spec(body=q, reference=ref)
    row = max(dve_ops._SUB_OPCODE_FOR_NAME.values()) + 1
    assert row < 0x20
    dve_ops._SUB_OPCODE_FOR_NAME[NAME] = row
    uops = lower(spec, ver="v3")
    sha = DveOpSpec(name=NAME, opcode=row, uops=uops,
                    rd1_en=_has_src1(spec)).sha("v3")
    op = dve_ops.DveOp(NAME, spec, subdim=False, uops_sha={"v3": sha})
    dve_ops.OPS.append(op)
    dve_ops.CUSTOM_DVE_SPECS[NAME] = spec
    return op


def build_nc(n_cores=N_CORES):
    from contextlib import ExitStack

    import concourse.bass as bass
    from concourse import mybir

    fp4_op = register_fp4_op()

    f32 = mybir.dt.float32
    f16 = mybir.dt.float16
    f8 = mybir.dt.float8e4

    nc = bass.Bass(num_devices=n_cores, debug=False)
    x_ext = nc.declare_dram_parameter("x", [P, L], f32, isOutput=False)
    out_ext = nc.declare_dram_parameter("out", [P, L], f16, isOutput=True)
    cc_in = nc.dram_tensor("cc_in", [1, 128], f32)
    cc_out = nc.dram_tensor("cc_out", [1, 128], f32, addr_space="Shared")
    cc_warm_in = nc.dram_tensor("cc_warm_in", [1, 128], f32)
    cc_warm_out = nc.dram_tensor("cc_warm_out", [1, 128], f32,
                                 addr_space="Shared")

    def act_reciprocal(act, out, in_):
        return act.add_instruction(
            mybir.InstActivation(
                name=act.bass.get_next_instruction_name(),
                func=mybir.ActivationFunctionType.Reciprocal,
                ins=[
                    act.lower_ap(in_),
                    mybir.ImmediateValue(dtype=f32, value=0.0),
                    mybir.ImmediateValue(dtype=f32, value=1.0),
                    mybir.ImmediateValue(dtype=f32, value=0.0),
                ],
                outs=[act.lower_ap(out)],
            )
        )

    with ExitStack() as ctx:
        def sem(name):
            return ctx.enter_context(nc.semaphore(name))

        def sbuf(name, shape, dt=f32):
            return ctx.enter_context(nc.sbuf_tensor(name, shape, dt))

        s_xa = [sem(f"s_xa{i}") for i in range(N_XA)]
        s_dve = sem("s_dve")
        s_conv = sem("s_conv")   # +1 per ACT conv (count = t+1)
        s_rm = sem("s_rm")       # +1 per ACT rm quarter (count = q+1)
        s_c16 = sem("s_c16")     # +1 per ACT c16 chunk (count = c+1)
        s_rs = sem("s_rs")       # +1 per ACT rs chunk (count = c+1)
        s_nf = sem("s_nf")       # +1 per ACT nicfull tile (count = t+1)
        s_cdma = sem("s_cdma")
        s_cc = sem("s_cc")
        s_warm = sem("s_warm")
        s_out = sem("s_out")
        s_ps = sem("s_ps")       # pool memset staging

        xh = sbuf("xh", [P, T * F], f16)            # 16MB: cache + output
        xa = [sbuf(f"xa{i}", [P, F]) for i in range(N_XA)]
        q16 = [sbuf(f"q16_{i}", [P, F], f16) for i in range(N_Q)]
        nf16 = [sbuf(f"nf16_{i}", [P, F], f16) for i in range(N_NF)]
        m_t = sbuf("m_t", [P, NBLK])                # blockmax -> 1/m -> s_b
        rs2 = [sbuf(f"rs2_{i}", [P, 1024], f16) for i in range(2)]
        f8_t = sbuf("f8_t", [P, 1024], f8)
        c16r = [sbuf(f"c16r_{i}", [P, 1024], f16) for i in range(2)]
        gall_t = sbuf("gall_t", [P, 128])
        mxq_t = sbuf("mxq_t", [P, NQ])
        mx_t = sbuf("mx_t", [P, 1])
        g128_t = sbuf("g128_t", [P, 1])
        st_t = sbuf("st_t", [P, 1])
        rt_t = sbuf("rt_t", [P, 1])
        k6_t = sbuf("k6_t", [P, 1])
        emask_t = sbuf("emask_t", [P, 1])

        dveA = [0] * T            # s_dve after reduce(t)
        K_mxq = [0] * NQ
        K_mx = [0]
        K_sb = [0] * NCH
        tag_q = [0] * T           # s_dve after op1(t)
        CH_LAST_TILE = [
            (CH_START[c] + CHUNKS[c]) // FBLK - 1 for c in range(NCH)]
        tag_o_dve = [0] * T       # s_dve after o(t) (dve-owned)

        def b3(ap):
            return ap.rearrange("p (b s) -> p b s", s=16)

        def qs(q):
            return slice(q * QBLK, (q + 1) * QBLK)

        def xs(t):
            return slice(t * F, (t + 1) * F)

        def cslice(c):
            return slice(CH_START[c], CH_START[c] + CHUNKS[c])

        def bcast(tens, t):
            bsl = slice(t * FBLK, (t + 1) * FBLK)
            return tens[:, bsl].unsqueeze(-1).broadcast_to([P, FBLK, 16])

        with nc.Block() as block:

            @block.vector
            def _(dve):
                cnt = 0

                def tag(ins):
                    nonlocal cnt
                    ins.then_inc(s_dve)
                    cnt += 1
                    return cnt

                k_ms = tag(dve.memset(emask_t[:], float("inf")))
                # custom-op warmup: first use pays a large one-time cost;
                # run a tiny dummy now so it lands off the critical path
                dve.wait_ge(s_dve, k_ms)
                tag(dve._custom_dve(
                    fp4_op,
                    out=q16[0][:, 0:16].rearrange("p (b s) -> p b s", s=16),
                    in0=xh[:, 0:16].rearrange("p (b s) -> p b s", s=16),
                    in1=c16r[0][:, 0:1].unsqueeze(-1).broadcast_to([P, 1, 16]),
                    s0=emask_t[:],
                    s1=MAGIC,
                ))

                # ---- pass A: per-block abs max (fp16, from the xh cache;
                # keeps the xa ring gated only on the ACT convert) ----
                for t in range(T):
                    dve.wait_ge(s_conv, t + 1)
                    dveA[t] = tag(dve.tensor_reduce(
                        out=m_t[:, t * FBLK:(t + 1) * FBLK],
                        in_=b3(xh[:, xs(t)]),
                        axis=mybir.AxisListType.X,
                        op=mybir.AluOpType.max,
                        apply_absolute_value=True,
                    ))
                    if (t + 1) % TQ == 0:
                        q = t // TQ
                        dve.wait_ge(s_dve, dveA[t])
                        K_mxq[q] = tag(dve.tensor_reduce(
                            out=mxq_t[:, q:q + 1], in_=m_t[:, qs(q)],
                            axis=mybir.AxisListType.X,
                            op=mybir.AluOpType.max,
                        ))
                dve.wait_ge(s_dve, K_mxq[NQ - 1])
                K_mx[0] = tag(dve.tensor_reduce(
                    out=mx_t[:], in_=mxq_t[:], axis=mybir.AxisListType.X,
                    op=mybir.AluOpType.max,
                ))

                # ---- global scalars (post-AllReduce) ----
                dve.wait_ge(s_cdma, 32)
                k_g = tag(dve.tensor_reduce(
                    out=g128_t[:], in_=gall_t[:], axis=mybir.AxisListType.X,
                    op=mybir.AluOpType.max))
                dve.wait_ge(s_dve, k_g)
                k_st = tag(dve.tensor_scalar(
                    st_t[:], g128_t[:], 1.0 / 2688.0, None,
                    op0=mybir.AluOpType.mult))
                dve.wait_ge(s_dve, k_st)
                tag(dve.tensor_scalar(
                    k6_t[:], st_t[:], 6.0, None, op0=mybir.AluOpType.mult))
                k_rt = tag(dve.reciprocal(rt_t[:], st_t[:]))

                # ---- per-block scale chain: f8 roundtrip per chunk ----
                # (rm = 1/m already in m_t from ACT, per quarter)
                def emit_chunk(c):
                    cs = cslice(c)
                    n = CHUNKS[c]
                    dve.wait_ge(s_rm, RMQ[c])
                    dve.wait_ge(s_dve, k_rt if c == 0 else K_sb[c - 1])
                    k_f8 = tag(dve.tensor_scalar(
                        f8_t[:, 0:n], m_t[:, cs], k6_t[:], None,
                        op0=mybir.AluOpType.mult))
                    dve.wait_ge(s_dve, k_f8)
                    K_sb[c] = tag(dve.tensor_copy(m_t[:, cs], f8_t[:, 0:n]))

                emit_chunk(0)
                emit_chunk(1)

                # ---- pass B: op1 (custom, bcast in1) + o (dense, 2x) ----
                next_chunk = 2
                for t in range(T):
                    # early chunks: all f8/sb done by tile 5
                    while (next_chunk < NCH
                           and t >= 2 * (next_chunk - 2) + 1):
                        emit_chunk(next_chunk)
                        next_chunk += 1
                    c = chunk_of_tile(t)
                    lo = t * FBLK - CH_START[c]
                    dve.wait_ge(s_c16, c + 1)
                    tag_q[t] = tag(dve._custom_dve(
                        fp4_op,
                        out=b3(q16[t % N_Q][:]),
                        in0=b3(xh[:, xs(t)]),
                        in1=c16r[c % 2][:, lo:lo + FBLK].unsqueeze(-1)
                            .broadcast_to([P, FBLK, 16]),
                        s0=emask_t[:],
                        s1=MAGIC,
                    ))
                    dve.wait_ge(s_nf, t + 1)
                    dve.wait_ge(s_dve, tag_q[t])
                    tag_o_dve[t] = tag(dve.tensor_tensor(
                        xh[:, xs(t)], q16[t % N_Q][:], nf16[t % N_NF][:],
                        op=mybir.AluOpType.mult))

            @block.scalar
            def _(act):
                # pass A: fp32 -> fp16 conversions into the cache;
                # rm = 1/m per quarter as its partial max completes
                for t in range(T):
                    act.wait_ge(s_xa[t % N_XA], 16 * (t // N_XA + 1))
                    act.activation(
                        xh[:, xs(t)], xa[t % N_XA][:],
                        mybir.ActivationFunctionType.Copy,
                    ).then_inc(s_conv)
                    if (t + 1) % TQ == 0:
                        q = t // TQ
                        act.wait_ge(s_dve, K_mxq[q])
                        act_reciprocal(
                            act, m_t[:, qs(q)], m_t[:, qs(q)]).then_inc(s_rm)

                # post-AR scale chain per chunk: c16 = rt*sb -> fp16,
                # rs = 1/sb; then nicfull per tile straight from the rs
                # chunk (dense bcast materialization, scaled by st, so the
                # o-multiply runs 2x)
                def chain(c):
                    cs = cslice(c)
                    n = CHUNKS[c]
                    act.wait_ge(s_dve, K_sb[c])
                    if c >= 2:
                        act.wait_ge(s_dve, tag_q[CH_LAST_TILE[c - 2]])
                    act.activation(
                        c16r[c % 2][:, 0:n], m_t[:, cs],
                        mybir.ActivationFunctionType.Copy,
                        scale=rt_t[:],
                    ).then_inc(s_c16)
                    act_reciprocal(
                        act, rs2[c % 2][:, 0:n], m_t[:, cs]).then_inc(s_rs)

                def nf(t):
                    c = chunk_of_tile(t)
                    lo = t * FBLK - CH_START[c]
                    src_ap = rs2[c % 2][:, lo:lo + FBLK]
                    if t >= N_NF:
                        act.wait_ge(s_dve, tag_o_dve[t - N_NF])
                    act.wait_ge(s_rs, c + 1)
                    act.activation(
                        b3(nf16[t % N_NF][:]),
                        src_ap.unsqueeze(-1).broadcast_to([P, FBLK, 16]),
                        mybir.ActivationFunctionType.Copy,
                        scale=st_t[:],
                    ).then_inc(s_nf)

                chain(0)
                chain(1)
                nf(0)
                chain(2)
                for t in range(1, 8):
                    nf(t)
                chain(3)
                for t in range(8, 16):
                    nf(t)
                chain(4)
                for t in range(16, T):
                    nf(t)

            @block.gpsimd
            def _(pool):
                pool.memset(gall_t[0:1, :], 0.0).then_inc(s_ps)
                pool.wait_ge(s_ps, 1)
                pool.dma_start(out=cc_warm_in[:, :],
                               in_=gall_t[0:1, :]).then_inc(s_warm, 16)
                pool.wait_ge(s_warm, 16)
                pool.collective_compute(
                    "AllReduce",
                    mybir.AluOpType.max,
                    replica_groups=[list(range(n_cores))],
                    ins=[cc_warm_in.ap().opt()],
                    outs=[cc_warm_out.ap().opt()],
                ).then_inc(s_cc)
                pool.wait_ge(s_dve, K_mx[0])
                pool.dma_start(out=cc_in[:, :],
                               in_=mx_t[:, :]).then_inc(s_cdma, 16)
                pool.wait_ge(s_cdma, 16)
                pool.collective_compute(
                    "AllReduce",
                    mybir.AluOpType.max,
                    replica_groups=[list(range(n_cores))],
                    ins=[cc_in.ap().opt()],
                    outs=[cc_out.ap().opt()],
                ).then_inc(s_cc)

            @block.sync
            def _(sync):
                # pass A input DMAs (single queue; slot gated on consumers)
                for t in range(T):
                    if t >= N_XA:
                        sync.wait_ge(s_conv, t - N_XA + 1)
                    sync.dma_start(
                        out=xa[t % N_XA][:, :],
                        in_=x_ext[:, xs(t)],
                    ).then_inc(s_xa[t % N_XA], 16)
                # collective staging (cc_in is DMA'd by the pool queue)
                sync.wait_ge(s_cc, 2)
                sync.dma_start(
                    out=gall_t[:, :],
                    in_=cc_out.ap().broadcast_to([P, 128]),
                ).then_inc(s_cdma, 16)
                # pass B: output DMAs (fp16, from the dead xh slot)
                for t in range(T):
                    sync.wait_ge(s_dve, tag_o_dve[t])
                    sync.dma_start(
                        out=out_ext[:, xs(t)],
                        in_=xh[:, xs(t)],
                    ).then_inc(s_out, 16)
                sync.wait_ge(s_out, 16 * T)

    mybir.codegen_inst_isa_subclasses(nc)
    return nc


_CACHE = {}


def _get_nc():
    if "nc" not in _CACHE:
        _CACHE["nc"] = build_nc()
    return _CACHE["nc"]


def kernel(x: np.ndarray) -> np.ndarray:
    from concourse.bass_utils import run_bass_kernel_spmd

    x = np.asarray(x, dtype=np.float32)
    assert x.shape == FULL_SHAPE
    shards = x.reshape(N_CORES, P, L)
    in_maps = [{"x": np.ascontiguousarray(shards[i])} for i in range(N_CORES)]
    nc = _get_nc()
    res = run_bass_kernel_spmd(nc, in_maps, core_ids=list(range(N_CORES)))
    out = np.stack([np.asarray(r["out"], dtype=np.float32)
                    for r in res.results], axis=0)
    return out.reshape(FULL_SHAPE)
